# revision 1
# baseline (speedup 1.0000x reference)
"""GatedBlock kernel: data-parallel over 8 NeuronCores.

Shards the leading N axis of x (200000 rows -> 25000/core), replicates
the small per-irrep weights W0/W1/W2, computes the o3.Linear + gate
activation block on each core, and gathers the full [N, 896] output.
"""
import numpy as np
import jax
import jax.numpy as jnp
from functools import partial

N = 200000
MUL0, MUL1, MUL2 = 256, 128, 64
SCALARS, NGATES = 256, 128
MULH = 64
SILU_NORM = 1.6791
SIGMOID_NORM = 1.8484
NCORES = 8

INV0 = 1.0 / np.sqrt(MUL0)
INV1 = 1.0 / np.sqrt(MUL1)
INV2 = 1.0 / np.sqrt(MUL2)


@partial(jax.pmap, in_axes=(0, None, None, None))
def _block(x, W0, W1, W2):
    n = x.shape[0]
    x0 = x[:, :MUL0]
    x1 = x[:, MUL0:MUL0 + MUL1 * 3].reshape(n, MUL1, 3)
    x2 = x[:, MUL0 + MUL1 * 3:].reshape(n, MUL2, 5)

    y0 = (x0 @ W0) * INV0
    # einsum over the multiplicity axis; fold the (2l+1) component axis
    # into the row axis so each is a single dense matmul on-device.
    y1 = jnp.einsum('nmc,mk->nkc', x1, W1) * INV1
    y2 = jnp.einsum('nmc,mk->nkc', x2, W2) * INV2

    out_scalars = jax.nn.silu(y0[:, :SCALARS]) * SILU_NORM
    g = jax.nn.sigmoid(y0[:, SCALARS:]) * SIGMOID_NORM
    g1 = g[:, :MULH, None]
    g2 = g[:, MULH:, None]
    return jnp.concatenate(
        [out_scalars,
         (y1 * g1).reshape(n, MULH * 3),
         (y2 * g2).reshape(n, MULH * 5)],
        axis=1,
    )


def kernel(x, W0, W1, W2):
    x = np.asarray(x, dtype=np.float32)
    W0 = np.asarray(W0, dtype=np.float32)
    W1 = np.asarray(W1, dtype=np.float32)
    W2 = np.asarray(W2, dtype=np.float32)
    n = x.shape[0]
    shard = n // NCORES
    xs = x.reshape(NCORES, shard, x.shape[1])
    out = _block(xs, W0, W1, W2)
    return np.asarray(out).reshape(n, MULH * 5 + MULH * 3 + SCALARS).astype(np.float32)



# revision 8
# speedup vs baseline: 113062.3409x; 113062.3409x over previous
"""GatedBlock (o3.Linear + gate) Bass kernel, data-parallel over 8 NeuronCores.

Strategy
--------
- Shard the leading N axis of x: 200000 rows -> 25000/core (padded to
  25088 = 49 * 512 for uniform tiling). Weights replicated.
- Host-side prep: cast to bf16 and store x TRANSPOSED + de-interleaved as
  xt [960, 25088] per core, so the contraction (feature) axis lands on SBUF
  partitions with zero on-chip transposes:
      rows   0:256  = x0^T                       (l=0, K=256 -> 2 chunks)
      rows 256:640  = x1_c^T for c=0..2          (l=1, K=128 each)
      rows 640:960  = x2_c^T for c=0..4          (l=2, K=64 each)
  Path norms and the e3nn activation norm constants are folded into the
  weights on the host (W0*INV0, W1*INV1*SIGMOID_NORM, W2*INV2*SIGMOID_NORM).
- Device: per 128-row subtile, 10 bf16 matmuls (x^T stationary, W moving)
  into 2 PSUM banks; one sigmoid over [128,384] on ACT; silu recomposed as
  (z*SILU_NORM)*sigmoid(z) on DVE; gate muls with broadcast APs on DVE;
  bf16 [128,768] contiguous row-major store.
- Output upcast bf16 -> f32 on host.
"""
import numpy as np
import ml_dtypes
from contextlib import ExitStack

import concourse.bass as bass
import concourse.tile as tile
from concourse import bacc, mybir
from concourse.bass_utils import run_bass_kernel_spmd

BF16 = mybir.dt.bfloat16
F32 = mybir.dt.float32
BFNP = ml_dtypes.bfloat16

N = 200000
NCORES = 8
R = N // NCORES            # 25000 rows per core
MACRO = 512                # rows per macro-tile
RP = 25088                 # padded rows per core (49 * 512)
OUTD = 768                 # 256 scalars + 64*3 + 64*5

SILU_NORM = 1.6791
SIGMOID_NORM = 1.8484
INV0 = 1.0 / np.sqrt(256.0)
INV1 = 1.0 / np.sqrt(128.0)
INV2 = 1.0 / np.sqrt(64.0)

_NC = None


def _build_nc():
    nc = bacc.Bacc("TRN2", target_bir_lowering=False, debug=False,
                   num_devices=NCORES)
    xt = nc.dram_tensor("xt", [960, RP], BF16, kind="ExternalInput").ap()
    w0 = nc.dram_tensor("w0", [256, 384], BF16, kind="ExternalInput").ap()
    w1 = nc.dram_tensor("w1", [128, 64], BF16, kind="ExternalInput").ap()
    w2 = nc.dram_tensor("w2", [64, 64], BF16, kind="ExternalInput").ap()
    out = nc.dram_tensor("out", [RP, OUTD], BF16, kind="ExternalOutput").ap()

    # DRAM view [partition, chunk, row] of feature-chunks 0..6 (features 0..895)
    xa_view = xt[0:896, :].rearrange("(k p) n -> p k n", p=128)

    with tile.TileContext(nc) as tc, ExitStack() as ctx:
        weights = ctx.enter_context(tc.tile_pool(name="weights", bufs=1))
        xin = ctx.enter_context(tc.tile_pool(name="xin", bufs=3))
        psum = ctx.enter_context(tc.tile_pool(name="psum", bufs=4, space="PSUM"))
        outp = ctx.enter_context(tc.tile_pool(name="outp", bufs=6))
        gpool = ctx.enter_context(tc.tile_pool(name="gpool", bufs=6))

        w0t = weights.tile([128, 2, 384], BF16)
        nc.sync.dma_start(out=w0t, in_=w0.rearrange("(a p) n -> p a n", p=128))
        w1t = weights.tile([128, 64], BF16)
        nc.sync.dma_start(out=w1t, in_=w1)
        # w2d: [128,128] block-diagonal (two copies of W2 on the diagonal).
        # The two x2 c-chunks sharing one 128-partition group then become a
        # single full-K=128 matmul each — K=64 matmuls at partition offsets
        # 0 and 64 would put the PE in conflicting split-array tile
        # positions, which crashes the exec unit.
        w2d = weights.tile([128, 128], BF16)
        nc.vector.memset(w2d, 0.0)
        nc.sync.dma_start(out=w2d[0:64, 0:64], in_=w2)
        nc.sync.dma_start(out=w2d[64:128, 64:128], in_=w2)
        w2t = weights.tile([64, 64], BF16)
        nc.sync.dma_start(out=w2t, in_=w2)

        for t in range(RP // MACRO):
            r0 = t * MACRO
            xa = xin.tile([128, 7, MACRO], BF16, tag="xa")
            nc.sync.dma_start(out=xa, in_=xa_view[:, :, r0:r0 + MACRO])
            xb = xin.tile([64, MACRO], BF16, tag="xb")   # x2 c=4 (features 896:960)
            nc.sync.dma_start(out=xb, in_=xt[896:960, r0:r0 + MACRO])

            for s in range(4):
                sub = slice(s * 128, (s + 1) * 128)
                y0 = psum.tile([128, 384], F32, tag="y0")
                nc.tensor.matmul(y0, xa[:, 0, sub], w0t[:, 0, :], start=True, stop=False)
                nc.tensor.matmul(y0, xa[:, 1, sub], w0t[:, 1, :], start=False, stop=True)
                y12 = psum.tile([128, 512], F32, tag="y12")
                for c in range(3):
                    nc.tensor.matmul(y12[:, c * 64:(c + 1) * 64], xa[:, 2 + c, sub],
                                     w1t, start=True, stop=True)
                for pair in range(2):
                    nc.tensor.matmul(y12[:, 192 + pair * 128:320 + pair * 128],
                                     xa[:, 5 + pair, sub], w2d,
                                     start=True, stop=True)
                nc.tensor.matmul(y12[:, 448:512], xb[:, sub], w2t,
                                 start=True, stop=True)

                # sigmoid over all 384 cols: 0:256 feeds the silu recomposition
                # z*sigmoid(z); 256:384 are the gates.
                sg = gpool.tile([128, 384], BF16, tag="sg")
                nc.scalar.activation(out=sg, in_=y0,
                                     func=mybir.ActivationFunctionType.Sigmoid)

                ot = outp.tile([128, OUTD], BF16, tag="ot")
                # out_scalars = SILU_NORM*silu(z) = (z*SILU_NORM)*sigmoid(z)
                nc.vector.scalar_tensor_tensor(ot[:, 0:256], y0[:, 0:256], SILU_NORM,
                                               sg[:, 0:256], mybir.AluOpType.mult,
                                               mybir.AluOpType.mult)
                o1 = ot[:, 256:448].rearrange("p (k c) -> p c k", c=3)
                y1v = y12[:, 0:192].rearrange("p (c k) -> p c k", k=64)
                nc.vector.tensor_tensor(
                    o1, y1v, sg[:, 256:320].unsqueeze(1).to_broadcast([128, 3, 64]),
                    mybir.AluOpType.mult)
                o2 = ot[:, 448:768].rearrange("p (k c) -> p c k", c=5)
                y2v = y12[:, 192:512].rearrange("p (c k) -> p c k", k=64)
                nc.vector.tensor_tensor(
                    o2, y2v, sg[:, 320:384].unsqueeze(1).to_broadcast([128, 5, 64]),
                    mybir.AluOpType.mult)
                nc.sync.dma_start(out=out[r0 + s * 128: r0 + (s + 1) * 128, :], in_=ot)
    nc.compile()
    return nc


def _get_nc():
    global _NC
    if _NC is None:
        _NC = _build_nc()
    return _NC


def _pack_xt(xc_bf: np.ndarray) -> np.ndarray:
    """xc_bf: [R, 960] bf16 -> [960, RP] bf16 transposed + de-interleaved."""
    n = xc_bf.shape[0]
    xt = np.zeros((960, RP), dtype=BFNP)
    xt[0:256, :n] = xc_bf[:, 0:256].T
    xt[256:640, :n] = (xc_bf[:, 256:640].reshape(n, 128, 3)
                       .transpose(2, 1, 0).reshape(384, n))
    xt[640:960, :n] = (xc_bf[:, 640:960].reshape(n, 64, 5)
                       .transpose(2, 1, 0).reshape(320, n))
    return xt


def _prepare_in_maps(x, W0, W1, W2):
    x = np.asarray(x)
    xbf = x.astype(BFNP)
    w0s = (np.asarray(W0, dtype=np.float32) * INV0).astype(BFNP)
    w1s = (np.asarray(W1, dtype=np.float32) * (INV1 * SIGMOID_NORM)).astype(BFNP)
    w2s = (np.asarray(W2, dtype=np.float32) * (INV2 * SIGMOID_NORM)).astype(BFNP)
    return [{"xt": _pack_xt(xbf[c * R:(c + 1) * R]), "w0": w0s, "w1": w1s,
             "w2": w2s} for c in range(NCORES)]


def _gather_out(results):
    out = np.empty((N, OUTD), dtype=np.float32)
    for c in range(NCORES):
        out[c * R:(c + 1) * R] = results[c]["out"][:R].astype(np.float32)
    return out


def kernel(x, W0, W1, W2):
    in_maps = _prepare_in_maps(x, W0, W1, W2)
    res = run_bass_kernel_spmd(_get_nc(), in_maps, core_ids=list(range(NCORES)))
    return _gather_out(res.results)


# revision 10
# speedup vs baseline: 114755.4820x; 1.0150x over previous
"""GatedBlock (o3.Linear + gate) Bass kernel, data-parallel over 8 NeuronCores.

Strategy
--------
- Shard the leading N axis of x: 200000 rows -> 25000/core (padded to
  25088 = 49 * 512 for uniform tiling). Weights replicated.
- Host-side prep: cast to bf16 and store x TRANSPOSED + de-interleaved as
  xt [960, 25088] per core, so the contraction (feature) axis lands on SBUF
  partitions with zero on-chip transposes:
      rows   0:256  = x0^T                       (l=0, K=256 -> 2 chunks)
      rows 256:640  = x1_c^T for c=0..2          (l=1, K=128 each)
      rows 640:960  = x2_c^T for c=0..4          (l=2, K=64 each)
  Path norms and the e3nn activation norm constants are folded into the
  weights on the host (W0*INV0, W1*INV1*SIGMOID_NORM, W2*INV2*SIGMOID_NORM).
- Device: per 128-row subtile, 10 bf16 matmuls (x^T stationary, W moving)
  into 2 PSUM banks; one sigmoid over [128,384] on ACT; silu recomposed as
  (z*SILU_NORM)*sigmoid(z) on DVE; gate muls with broadcast APs on DVE;
  bf16 [128,768] contiguous row-major store.
- Output upcast bf16 -> f32 on host.
"""
import numpy as np
import ml_dtypes
from contextlib import ExitStack

import concourse.bass as bass
import concourse.tile as tile
from concourse import bacc, mybir
from concourse.bass_utils import run_bass_kernel_spmd

BF16 = mybir.dt.bfloat16
F32 = mybir.dt.float32
BFNP = ml_dtypes.bfloat16

N = 200000
NCORES = 8
R = N // NCORES            # 25000 rows per core
MACRO = 512                # rows per macro-tile
RP = 25088                 # padded rows per core (49 * 512)
OUTD = 768                 # 256 scalars + 64*3 + 64*5

SILU_NORM = 1.6791
SIGMOID_NORM = 1.8484
INV0 = 1.0 / np.sqrt(256.0)
INV1 = 1.0 / np.sqrt(128.0)
INV2 = 1.0 / np.sqrt(64.0)

_NC = None


def _build_nc():
    nc = bacc.Bacc("TRN2", target_bir_lowering=False, debug=False,
                   num_devices=NCORES)
    xt = nc.dram_tensor("xt", [960, RP], BF16, kind="ExternalInput").ap()
    w0 = nc.dram_tensor("w0", [256, 384], BF16, kind="ExternalInput").ap()
    w1 = nc.dram_tensor("w1", [128, 64], BF16, kind="ExternalInput").ap()
    w2 = nc.dram_tensor("w2", [64, 64], BF16, kind="ExternalInput").ap()
    out = nc.dram_tensor("out", [RP, OUTD], BF16, kind="ExternalOutput").ap()

    # DRAM view [partition, chunk, row] of feature-chunks 0..6 (features 0..895)
    xa_view = xt[0:896, :].rearrange("(k p) n -> p k n", p=128)

    with tile.TileContext(nc) as tc, ExitStack() as ctx:
        weights = ctx.enter_context(tc.tile_pool(name="weights", bufs=1))
        xin = ctx.enter_context(tc.tile_pool(name="xin", bufs=3))
        psum = ctx.enter_context(tc.tile_pool(name="psum", bufs=4, space="PSUM"))
        outp = ctx.enter_context(tc.tile_pool(name="outp", bufs=6))
        gpool = ctx.enter_context(tc.tile_pool(name="gpool", bufs=6))

        w0t = weights.tile([128, 2, 384], BF16)
        nc.sync.dma_start(out=w0t, in_=w0.rearrange("(a p) n -> p a n", p=128))
        w1t = weights.tile([128, 64], BF16)
        nc.sync.dma_start(out=w1t, in_=w1)
        # w2d: [128,128] block-diagonal (two copies of W2 on the diagonal).
        # The two x2 c-chunks sharing one 128-partition group then become a
        # single full-K=128 matmul each — K=64 matmuls at partition offsets
        # 0 and 64 would put the PE in conflicting split-array tile
        # positions, which crashes the exec unit.
        w2d = weights.tile([128, 128], BF16)
        nc.vector.memset(w2d, 0.0)
        nc.sync.dma_start(out=w2d[0:64, 0:64], in_=w2)
        nc.sync.dma_start(out=w2d[64:128, 64:128], in_=w2)
        w2t = weights.tile([64, 64], BF16)
        nc.sync.dma_start(out=w2t, in_=w2)

        for t in range(RP // MACRO):
            r0 = t * MACRO
            xa = xin.tile([128, 7, MACRO], BF16, tag="xa")
            nc.sync.dma_start(out=xa, in_=xa_view[:, :, r0:r0 + MACRO])
            xb = xin.tile([64, MACRO], BF16, tag="xb")   # x2 c=4 (features 896:960)
            nc.sync.dma_start(out=xb, in_=xt[896:960, r0:r0 + MACRO])

            for s in range(4):
                sub = slice(s * 128, (s + 1) * 128)
                y0 = psum.tile([128, 384], F32, tag="y0")
                nc.tensor.matmul(y0, xa[:, 0, sub], w0t[:, 0, :], start=True, stop=False)
                nc.tensor.matmul(y0, xa[:, 1, sub], w0t[:, 1, :], start=False, stop=True)
                y12 = psum.tile([128, 512], F32, tag="y12")
                for c in range(3):
                    nc.tensor.matmul(y12[:, c * 64:(c + 1) * 64], xa[:, 2 + c, sub],
                                     w1t, start=True, stop=True)
                for pair in range(2):
                    nc.tensor.matmul(y12[:, 192 + pair * 128:320 + pair * 128],
                                     xa[:, 5 + pair, sub], w2d,
                                     start=True, stop=True)
                nc.tensor.matmul(y12[:, 448:512], xb[:, sub], w2t,
                                 start=True, stop=True)

                # sigmoid over all 384 cols: 0:256 feeds the silu recomposition
                # z*sigmoid(z); 256:384 are the gates.
                sg = gpool.tile([128, 384], BF16, tag="sg")
                nc.scalar.activation(out=sg, in_=y0,
                                     func=mybir.ActivationFunctionType.Sigmoid)

                ot = outp.tile([128, OUTD], BF16, tag="ot")
                # out_scalars = SILU_NORM*silu(z) = (z*SILU_NORM)*sigmoid(z)
                nc.vector.scalar_tensor_tensor(ot[:, 0:256], y0[:, 0:256], SILU_NORM,
                                               sg[:, 0:256], mybir.AluOpType.mult,
                                               mybir.AluOpType.mult)
                # Gating into BLOCKED (c-major) layout — contiguous DVE writes;
                # the host de-interleaves to the reference (k-major) layout.
                o1 = ot[:, 256:448].rearrange("p (c k) -> p c k", k=64)
                y1v = y12[:, 0:192].rearrange("p (c k) -> p c k", k=64)
                nc.vector.tensor_tensor(
                    o1, y1v, sg[:, 256:320].unsqueeze(1).to_broadcast([128, 3, 64]),
                    mybir.AluOpType.mult)
                o2 = ot[:, 448:768].rearrange("p (c k) -> p c k", k=64)
                y2v = y12[:, 192:512].rearrange("p (c k) -> p c k", k=64)
                nc.vector.tensor_tensor(
                    o2, y2v, sg[:, 320:384].unsqueeze(1).to_broadcast([128, 5, 64]),
                    mybir.AluOpType.mult)
                nc.sync.dma_start(out=out[r0 + s * 128: r0 + (s + 1) * 128, :], in_=ot)
    nc.compile()
    return nc


def _get_nc():
    global _NC
    if _NC is None:
        _NC = _build_nc()
    return _NC


def _pack_xt(xc_bf: np.ndarray) -> np.ndarray:
    """xc_bf: [R, 960] bf16 -> [960, RP] bf16 transposed + de-interleaved."""
    n = xc_bf.shape[0]
    xt = np.zeros((960, RP), dtype=BFNP)
    xt[0:256, :n] = xc_bf[:, 0:256].T
    xt[256:640, :n] = (xc_bf[:, 256:640].reshape(n, 128, 3)
                       .transpose(2, 1, 0).reshape(384, n))
    xt[640:960, :n] = (xc_bf[:, 640:960].reshape(n, 64, 5)
                       .transpose(2, 1, 0).reshape(320, n))
    return xt


def _prepare_in_maps(x, W0, W1, W2):
    x = np.asarray(x)
    xbf = x.astype(BFNP)
    w0s = (np.asarray(W0, dtype=np.float32) * INV0).astype(BFNP)
    w1s = (np.asarray(W1, dtype=np.float32) * (INV1 * SIGMOID_NORM)).astype(BFNP)
    w2s = (np.asarray(W2, dtype=np.float32) * (INV2 * SIGMOID_NORM)).astype(BFNP)
    return [{"xt": _pack_xt(xbf[c * R:(c + 1) * R]), "w0": w0s, "w1": w1s,
             "w2": w2s} for c in range(NCORES)]


def _gather_out(results):
    out = np.empty((N, OUTD), dtype=np.float32)
    for c in range(NCORES):
        o = results[c]["out"][:R].astype(np.float32)
        dst = out[c * R:(c + 1) * R]
        dst[:, 0:256] = o[:, 0:256]
        # device wrote gated l=1/l=2 blocks c-major; reference is k-major
        dst[:, 256:448] = (o[:, 256:448].reshape(R, 3, 64)
                           .transpose(0, 2, 1).reshape(R, 192))
        dst[:, 448:768] = (o[:, 448:768].reshape(R, 5, 64)
                           .transpose(0, 2, 1).reshape(R, 320))
    return out


def kernel(x, W0, W1, W2):
    in_maps = _prepare_in_maps(x, W0, W1, W2)
    res = run_bass_kernel_spmd(_get_nc(), in_maps, core_ids=list(range(NCORES)))
    return _gather_out(res.results)


# revision 14
# speedup vs baseline: 124971.7538x; 1.0890x over previous
"""GatedBlock (o3.Linear + gate) Bass kernel, data-parallel over 8 NeuronCores.

Strategy
--------
- Shard the leading N axis of x: 200000 rows -> 25000/core (padded to
  25088 = 49 * 512 for uniform tiling). Weights replicated.
- Host-side prep: cast to bf16 and store x TRANSPOSED + de-interleaved as
  xt [960, 25088] per core, so the contraction (feature) axis lands on SBUF
  partitions with zero on-chip transposes:
      rows   0:256  = x0^T                       (l=0, K=256 -> 2 chunks)
      rows 256:640  = x1_c^T for c=0..2          (l=1, K=128 each)
      rows 640:960  = x2_c^T for c=0..4          (l=2, K=64 each)
  Path norms and the e3nn activation norm constants are folded into the
  weights on the host (W0*INV0, W1*INV1*SIGMOID_NORM, W2*INV2*SIGMOID_NORM).
- Device: per 128-row subtile, 10 bf16 matmuls (x^T stationary, W moving)
  into 2 PSUM banks; one sigmoid over [128,384] on ACT; silu recomposed as
  (z*SILU_NORM)*sigmoid(z) on DVE; gate muls with broadcast APs on DVE;
  bf16 [128,768] contiguous row-major store.
- Output upcast bf16 -> f32 on host.
"""
import numpy as np
import ml_dtypes
from contextlib import ExitStack

import concourse.bass as bass
import concourse.tile as tile
from concourse import bacc, mybir
from concourse.bass_utils import run_bass_kernel_spmd

BF16 = mybir.dt.bfloat16
F32 = mybir.dt.float32
BFNP = ml_dtypes.bfloat16

N = 200000
NCORES = 8
R = N // NCORES            # 25000 rows per core
MACRO = 512                # rows per macro-tile
RP = 25088                 # padded rows per core (49 * 512)
OUTD = 768                 # 256 scalars + 64*3 + 64*5

SILU_NORM = 1.6791
SIGMOID_NORM = 1.8484
INV0 = 1.0 / np.sqrt(256.0)
INV1 = 1.0 / np.sqrt(128.0)
INV2 = 1.0 / np.sqrt(64.0)

_NC = None


def _build_nc(rp=RP):
    nc = bacc.Bacc("TRN2", target_bir_lowering=False, debug=False,
                   num_devices=NCORES)
    xt = nc.dram_tensor("xt", [960, rp], BF16, kind="ExternalInput").ap()
    w0 = nc.dram_tensor("w0", [256, 384], BF16, kind="ExternalInput").ap()
    w1 = nc.dram_tensor("w1", [128, 64], BF16, kind="ExternalInput").ap()
    w2 = nc.dram_tensor("w2", [64, 64], BF16, kind="ExternalInput").ap()
    out = nc.dram_tensor("out", [rp, OUTD], BF16, kind="ExternalOutput").ap()

    # DRAM view [partition, chunk, row] of feature-chunks 0..6 (features 0..895)
    xa_view = xt[0:896, :].rearrange("(k p) n -> p k n", p=128)

    with tile.TileContext(nc) as tc, ExitStack() as ctx:
        weights = ctx.enter_context(tc.tile_pool(name="weights", bufs=1))
        xin = ctx.enter_context(tc.tile_pool(name="xin", bufs=3))
        psum = ctx.enter_context(tc.tile_pool(name="psum", bufs=2, space="PSUM"))
        outp = ctx.enter_context(tc.tile_pool(name="outp", bufs=6))
        gpool = ctx.enter_context(tc.tile_pool(name="gpool", bufs=6))

        w0t = weights.tile([128, 2, 384], BF16)
        nc.sync.dma_start(out=w0t, in_=w0.rearrange("(a p) n -> p a n", p=128))
        w1t = weights.tile([128, 64], BF16)
        nc.sync.dma_start(out=w1t, in_=w1)
        # w2d: [128,128] block-diagonal (two copies of W2 on the diagonal).
        # The two x2 c-chunks sharing one 128-partition group then become a
        # single full-K=128 matmul each — K=64 matmuls at partition offsets
        # 0 and 64 would put the PE in conflicting split-array tile
        # positions, which crashes the exec unit.
        w2d = weights.tile([128, 128], BF16)
        nc.vector.memset(w2d, 0.0)
        nc.sync.dma_start(out=w2d[0:64, 0:64], in_=w2)
        nc.sync.dma_start(out=w2d[64:128, 64:128], in_=w2)
        w2t = weights.tile([64, 64], BF16)
        nc.sync.dma_start(out=w2t, in_=w2)

        for t in range(rp // MACRO):
            r0 = t * MACRO
            xa = xin.tile([128, 7, MACRO], BF16, tag="xa")
            nc.sync.dma_start(out=xa, in_=xa_view[:, :, r0:r0 + MACRO])
            xb = xin.tile([64, MACRO], BF16, tag="xb")   # x2 c=4 (features 896:960)
            nc.sync.dma_start(out=xb, in_=xt[896:960, r0:r0 + MACRO])

            # Process subtiles in PAIRS (u = 0,1 -> 256 rows per group): the
            # epilogue then runs one sigmoid / one PSUM->SBUF copy / one
            # scalar_tensor_tensor per group instead of per subtile, halving
            # per-op fixed overheads so the PSUM banks drain faster than the
            # PE refills them (keeps the PE out of low p-state stalls).
            for h in range(2):
                # y0d/y12d: [128, 2, 512] f32 = 2 PSUM banks, u-stride is one
                # bank so every matmul output stays inside a single bank.
                y0d = psum.tile([128, 2, 512], F32, tag="y0")
                y12d = psum.tile([128, 2, 512], F32, tag="y12")
                for u in range(2):
                    sub = slice((2 * h + u) * 128, (2 * h + u + 1) * 128)
                    y0 = y0d[:, u, :]
                    y12 = y12d[:, u, :]
                    nc.tensor.matmul(y0[:, 0:384], xa[:, 0, sub], w0t[:, 0, :],
                                     start=True, stop=False)
                    nc.tensor.matmul(y0[:, 0:384], xa[:, 1, sub], w0t[:, 1, :],
                                     start=False, stop=True)
                    for c in range(3):
                        nc.tensor.matmul(y12[:, c * 64:(c + 1) * 64],
                                         xa[:, 2 + c, sub], w1t,
                                         start=True, stop=True)
                    for pair in range(2):
                        nc.tensor.matmul(y12[:, 192 + pair * 128:320 + pair * 128],
                                         xa[:, 5 + pair, sub], w2d,
                                         start=True, stop=True)
                    nc.tensor.matmul(y12[:, 448:512], xb[:, sub], w2t,
                                     start=True, stop=True)

                # sigmoid over both subtiles' [*,384] in one ACT op: cols 0:256
                # feed the silu recomposition z*sigmoid(z); 256:384 = gates.
                sg = gpool.tile([128, 2, 384], BF16, tag="sg")
                nc.scalar.activation(out=sg, in_=y0d[:, :, 0:384],
                                     func=mybir.ActivationFunctionType.Sigmoid)
                # y12 PSUM -> SBUF bf16 via ACT (one op); gating then runs on
                # DVE in 16-bit 2x mode / on GpSimd (which cannot read PSUM).
                y12s = gpool.tile([128, 2, 512], BF16, tag="y12s")
                nc.scalar.copy(y12s, y12d)

                ot = outp.tile([128, 2, OUTD], BF16, tag="ot")
                # out_scalars = SILU_NORM*silu(z) = (z*SILU_NORM)*sigmoid(z)
                nc.vector.scalar_tensor_tensor(ot[:, :, 0:256], y0d[:, :, 0:256],
                                               SILU_NORM, sg[:, :, 0:256],
                                               mybir.AluOpType.mult,
                                               mybir.AluOpType.mult)
                # Gating in BLOCKED (c-major) layout — contiguous writes; the
                # host de-interleaves back to the reference k-major layout.
                for u in range(2):
                    nc.vector.tensor_tensor(
                        ot[:, u, 256:448].rearrange("p (c k) -> p c k", k=64),
                        y12s[:, u, 0:192].rearrange("p (c k) -> p c k", k=64),
                        sg[:, u, 256:320].unsqueeze(1).to_broadcast([128, 3, 64]),
                        mybir.AluOpType.mult)
                    nc.gpsimd.tensor_tensor(
                        ot[:, u, 448:768].rearrange("p (c k) -> p c k", k=64),
                        y12s[:, u, 192:512].rearrange("p (c k) -> p c k", k=64),
                        sg[:, u, 320:384].unsqueeze(1).to_broadcast([128, 5, 64]),
                        mybir.AluOpType.mult)
                rr = r0 + h * 256
                nc.sync.dma_start(
                    out=out[rr:rr + 256, :].rearrange("(u p) d -> p u d", p=128),
                    in_=ot)
    nc.compile()
    return nc


def _get_nc():
    global _NC
    if _NC is None:
        _NC = _build_nc()
    return _NC


def _pack_xt(xc_bf: np.ndarray) -> np.ndarray:
    """xc_bf: [R, 960] bf16 -> [960, RP] bf16 transposed + de-interleaved."""
    n = xc_bf.shape[0]
    xt = np.zeros((960, RP), dtype=BFNP)
    xt[0:256, :n] = xc_bf[:, 0:256].T
    xt[256:640, :n] = (xc_bf[:, 256:640].reshape(n, 128, 3)
                       .transpose(2, 1, 0).reshape(384, n))
    xt[640:960, :n] = (xc_bf[:, 640:960].reshape(n, 64, 5)
                       .transpose(2, 1, 0).reshape(320, n))
    return xt


def _prepare_in_maps(x, W0, W1, W2):
    x = np.asarray(x)
    xbf = x.astype(BFNP)
    w0s = (np.asarray(W0, dtype=np.float32) * INV0).astype(BFNP)
    w1s = (np.asarray(W1, dtype=np.float32) * (INV1 * SIGMOID_NORM)).astype(BFNP)
    w2s = (np.asarray(W2, dtype=np.float32) * (INV2 * SIGMOID_NORM)).astype(BFNP)
    return [{"xt": _pack_xt(xbf[c * R:(c + 1) * R]), "w0": w0s, "w1": w1s,
             "w2": w2s} for c in range(NCORES)]


def _gather_out(results):
    out = np.empty((N, OUTD), dtype=np.float32)
    for c in range(NCORES):
        o = results[c]["out"][:R].astype(np.float32)
        dst = out[c * R:(c + 1) * R]
        dst[:, 0:256] = o[:, 0:256]
        # device wrote gated l=1/l=2 blocks c-major; reference is k-major
        dst[:, 256:448] = (o[:, 256:448].reshape(R, 3, 64)
                           .transpose(0, 2, 1).reshape(R, 192))
        dst[:, 448:768] = (o[:, 448:768].reshape(R, 5, 64)
                           .transpose(0, 2, 1).reshape(R, 320))
    return out


def kernel(x, W0, W1, W2):
    in_maps = _prepare_in_maps(x, W0, W1, W2)
    res = run_bass_kernel_spmd(_get_nc(), in_maps, core_ids=list(range(NCORES)))
    return _gather_out(res.results)


# revision 18
# speedup vs baseline: 142230.5060x; 1.1381x over previous
"""GatedBlock (o3.Linear + gate) Bass kernel, data-parallel over 8 NeuronCores.

Strategy
--------
- Shard the leading N axis of x: 200000 rows -> 25000/core (padded to
  25088 = 49 * 512 for uniform tiling). Weights replicated.
- Host-side prep: cast to bf16 and store x TRANSPOSED + de-interleaved as
  xt [960, 25088] per core, so the contraction (feature) axis lands on SBUF
  partitions with zero on-chip transposes:
      rows   0:256  = x0^T                       (l=0, K=256 -> 2 chunks)
      rows 256:640  = x1_c^T for c=0..2          (l=1, K=128 each)
      rows 640:960  = x2_c^T for c=0..4          (l=2, K=64 each)
  Path norms and the e3nn activation norm constants are folded into the
  weights on the host (W0*INV0, W1*INV1*SIGMOID_NORM, W2*INV2*SIGMOID_NORM).
- Device: per 128-row subtile, 10 bf16 matmuls (x^T stationary, W moving)
  into 2 PSUM banks; one sigmoid over [128,384] on ACT; silu recomposed as
  (z*SILU_NORM)*sigmoid(z) on DVE; gate muls with broadcast APs on DVE;
  bf16 [128,768] contiguous row-major store.
- Output upcast bf16 -> f32 on host.
"""
import numpy as np
import ml_dtypes
from contextlib import ExitStack

import concourse.bass as bass
import concourse.tile as tile
from concourse import bacc, mybir
from concourse.bass_utils import run_bass_kernel_spmd

BF16 = mybir.dt.bfloat16
F32 = mybir.dt.float32
BFNP = ml_dtypes.bfloat16

N = 200000
NCORES = 8
R = N // NCORES            # 25000 rows per core
MACRO = 1024               # rows per macro-tile
RP = 25088                 # padded rows per core (49 * 512)
OUTD = 768                 # 256 scalars + 64*3 + 64*5

SILU_NORM = 1.6791
SIGMOID_NORM = 1.8484
INV0 = 1.0 / np.sqrt(256.0)
INV1 = 1.0 / np.sqrt(128.0)
INV2 = 1.0 / np.sqrt(64.0)

_NC = None


def _build_nc(rp=RP):
    nc = bacc.Bacc("TRN2", target_bir_lowering=False, debug=False,
                   num_devices=NCORES)
    xt = nc.dram_tensor("xt", [960, rp], BF16, kind="ExternalInput").ap()
    w0 = nc.dram_tensor("w0", [256, 384], BF16, kind="ExternalInput").ap()
    w1 = nc.dram_tensor("w1", [128, 64], BF16, kind="ExternalInput").ap()
    w2 = nc.dram_tensor("w2", [64, 64], BF16, kind="ExternalInput").ap()
    out = nc.dram_tensor("out", [rp, OUTD], BF16, kind="ExternalOutput").ap()

    # DRAM view [partition, chunk, row] of feature-chunks 0..6 (features 0..895)
    xa_view = xt[0:896, :].rearrange("(k p) n -> p k n", p=128)

    with tile.TileContext(nc) as tc, ExitStack() as ctx:
        weights = ctx.enter_context(tc.tile_pool(name="weights", bufs=1))
        xin = ctx.enter_context(tc.tile_pool(name="xin", bufs=3))
        psum = ctx.enter_context(tc.tile_pool(name="psum", bufs=2, space="PSUM"))
        outp = ctx.enter_context(tc.tile_pool(name="outp", bufs=6))
        gpool = ctx.enter_context(tc.tile_pool(name="gpool", bufs=6))

        w0t = weights.tile([128, 2, 384], BF16)
        nc.sync.dma_start(out=w0t, in_=w0.rearrange("(a p) n -> p a n", p=128))
        w1t = weights.tile([128, 64], BF16)
        nc.sync.dma_start(out=w1t, in_=w1)
        # w2d: [128,128] block-diagonal (two copies of W2 on the diagonal).
        # The two x2 c-chunks sharing one 128-partition group then become a
        # single full-K=128 matmul each — K=64 matmuls at partition offsets
        # 0 and 64 would put the PE in conflicting split-array tile
        # positions, which crashes the exec unit.
        w2d = weights.tile([128, 128], BF16)
        nc.vector.memset(w2d, 0.0)
        nc.sync.dma_start(out=w2d[0:64, 0:64], in_=w2)
        nc.sync.dma_start(out=w2d[64:128, 64:128], in_=w2)
        w2t = weights.tile([64, 64], BF16)
        nc.sync.dma_start(out=w2t, in_=w2)

        macros = [(i * MACRO, MACRO) for i in range(rp // MACRO)]
        if rp % MACRO:
            assert rp % MACRO % 256 == 0
            macros.append((rp - rp % MACRO, rp % MACRO))
        for r0, msz in macros:
            xa = xin.tile([128, 7, MACRO], BF16, tag="xa")
            nc.sync.dma_start(out=xa[:, :, 0:msz], in_=xa_view[:, :, r0:r0 + msz])
            xb = xin.tile([64, MACRO], BF16, tag="xb")   # x2 c=4 (features 896:960)
            nc.sync.dma_start(out=xb[:, 0:msz], in_=xt[896:960, r0:r0 + msz])

            # Process subtiles in PAIRS (u = 0,1 -> 256 rows per group): the
            # epilogue then runs one sigmoid / one PSUM->SBUF copy / one
            # scalar_tensor_tensor per group instead of per subtile, halving
            # per-op fixed overheads so the PSUM banks drain faster than the
            # PE refills them (keeps the PE out of low p-state stalls).
            for h in range(msz // 256):
                # y0d/y12d: [128, 2, 512] f32 = 2 PSUM banks, u-stride is one
                # bank so every matmul output stays inside a single bank.
                y0d = psum.tile([128, 2, 512], F32, tag="y0")
                y12d = psum.tile([128, 2, 512], F32, tag="y12")
                for u in range(2):
                    sub = slice((2 * h + u) * 128, (2 * h + u + 1) * 128)
                    y0 = y0d[:, u, :]
                    y12 = y12d[:, u, :]
                    nc.tensor.matmul(y0[:, 0:384], xa[:, 0, sub], w0t[:, 0, :],
                                     start=True, stop=False)
                    nc.tensor.matmul(y0[:, 0:384], xa[:, 1, sub], w0t[:, 1, :],
                                     start=False, stop=True)
                    for c in range(3):
                        nc.tensor.matmul(y12[:, c * 64:(c + 1) * 64],
                                         xa[:, 2 + c, sub], w1t,
                                         start=True, stop=True)
                    for pair in range(2):
                        nc.tensor.matmul(y12[:, 192 + pair * 128:320 + pair * 128],
                                         xa[:, 5 + pair, sub], w2d,
                                         start=True, stop=True)
                    nc.tensor.matmul(y12[:, 448:512], xb[:, sub], w2t,
                                     start=True, stop=True)

                # sigmoid over both subtiles' [*,384] in one ACT op: cols 0:256
                # feed the silu recomposition z*sigmoid(z); 256:384 = gates.
                sg = gpool.tile([128, 2, 384], BF16, tag="sg")
                nc.scalar.activation(out=sg, in_=y0d[:, :, 0:384],
                                     func=mybir.ActivationFunctionType.Sigmoid)
                # y12 PSUM -> SBUF bf16 via ACT (one op); gating then runs on
                # DVE in 16-bit 2x mode / on GpSimd (which cannot read PSUM).
                y12s = gpool.tile([128, 2, 512], BF16, tag="y12s")
                nc.scalar.copy(y12s, y12d)

                ot = outp.tile([128, 2, OUTD], BF16, tag="ot")
                # out_scalars = SILU_NORM*silu(z) = (z*SILU_NORM)*sigmoid(z)
                nc.vector.scalar_tensor_tensor(ot[:, :, 0:256], y0d[:, :, 0:256],
                                               SILU_NORM, sg[:, :, 0:256],
                                               mybir.AluOpType.mult,
                                               mybir.AluOpType.mult)
                # Gating in BLOCKED (c-major) layout — contiguous writes; the
                # host de-interleaves back to the reference k-major layout.
                for u in range(2):
                    nc.vector.tensor_tensor(
                        ot[:, u, 256:448].rearrange("p (c k) -> p c k", k=64),
                        y12s[:, u, 0:192].rearrange("p (c k) -> p c k", k=64),
                        sg[:, u, 256:320].unsqueeze(1).to_broadcast([128, 3, 64]),
                        mybir.AluOpType.mult)
                    nc.gpsimd.tensor_tensor(
                        ot[:, u, 448:768].rearrange("p (c k) -> p c k", k=64),
                        y12s[:, u, 192:512].rearrange("p (c k) -> p c k", k=64),
                        sg[:, u, 320:384].unsqueeze(1).to_broadcast([128, 5, 64]),
                        mybir.AluOpType.mult)
                rr = r0 + h * 256
                nc.sync.dma_start(
                    out=out[rr:rr + 256, :].rearrange("(u p) d -> p u d", p=128),
                    in_=ot)
    nc.compile()
    return nc


def _get_nc():
    global _NC
    if _NC is None:
        _NC = _build_nc()
    return _NC


def _pack_xt(xc_bf: np.ndarray) -> np.ndarray:
    """xc_bf: [R, 960] bf16 -> [960, RP] bf16 transposed + de-interleaved."""
    n = xc_bf.shape[0]
    xt = np.zeros((960, RP), dtype=BFNP)
    xt[0:256, :n] = xc_bf[:, 0:256].T
    xt[256:640, :n] = (xc_bf[:, 256:640].reshape(n, 128, 3)
                       .transpose(2, 1, 0).reshape(384, n))
    xt[640:960, :n] = (xc_bf[:, 640:960].reshape(n, 64, 5)
                       .transpose(2, 1, 0).reshape(320, n))
    return xt


def _prepare_in_maps(x, W0, W1, W2):
    x = np.asarray(x)
    xbf = x.astype(BFNP)
    w0s = (np.asarray(W0, dtype=np.float32) * INV0).astype(BFNP)
    w1s = (np.asarray(W1, dtype=np.float32) * (INV1 * SIGMOID_NORM)).astype(BFNP)
    w2s = (np.asarray(W2, dtype=np.float32) * (INV2 * SIGMOID_NORM)).astype(BFNP)
    return [{"xt": _pack_xt(xbf[c * R:(c + 1) * R]), "w0": w0s, "w1": w1s,
             "w2": w2s} for c in range(NCORES)]


def _gather_out(results):
    out = np.empty((N, OUTD), dtype=np.float32)
    for c in range(NCORES):
        o = results[c]["out"][:R].astype(np.float32)
        dst = out[c * R:(c + 1) * R]
        dst[:, 0:256] = o[:, 0:256]
        # device wrote gated l=1/l=2 blocks c-major; reference is k-major
        dst[:, 256:448] = (o[:, 256:448].reshape(R, 3, 64)
                           .transpose(0, 2, 1).reshape(R, 192))
        dst[:, 448:768] = (o[:, 448:768].reshape(R, 5, 64)
                           .transpose(0, 2, 1).reshape(R, 320))
    return out


def kernel(x, W0, W1, W2):
    in_maps = _prepare_in_maps(x, W0, W1, W2)
    res = run_bass_kernel_spmd(_get_nc(), in_maps, core_ids=list(range(NCORES)))
    return _gather_out(res.results)
